# revision 11
# baseline (speedup 1.0000x reference)
"""Trainium2 Bass kernel for nn_BlockDiagonalLayer.

Computes out[b, n*64+j] = sin(omega[n] * (sum_i x[b,n,i] * W[n,j,i] + bias[n,j]))
for B=2048, N=1024 networks, D_IN=D_OUT=64, sharded over 8 NeuronCores along N.

v3 strategy (per core, 128 networks = 64 pairs):
  - Host pre-scales W* = (omega/2pi)*W and b* = (omega/2pi)*bias so the PE
    directly produces u = arg/2pi in PSUM (period-1 phase).
  - x sent as 3 bytes/elem: xh = fp16(x) plus xl8 = fp8e4((x - xh) * 2^11).
  - W* sent as block-diagonal bf16 hi/lo pair (Wh, Wl) plus Whs = bf16(W*
    * 2^-11) for the x-lo pass. All matmuls 16/8-bit = 1 PE cycle/column:
      u = Wh^T xh + Wl^T xh + Whs^T xl8 + b* (bf16 hi/lo rows x ones)
  - Range reduction (3 elementwise ops):
      t   = Identity(u + MAGIC)            (ScalarE; t - MAGIC = rne(u))
      d   = (t - MAGIC) - u                (VectorE scalar_tensor_tensor,
                                            fp16 out; d = -frac in [-.5,.5])
      out = Sin(-2pi * d)                  (ScalarE, bf16 out)
  - Output stored transposed bf16 [pair, j2, b]; host transposes/upcasts.
"""

import numpy as np
import ml_dtypes

import concourse.bass as bass
import concourse.tile as tile
from concourse import bacc, mybir
from concourse.alu_op_type import AluOpType
from concourse.bass_utils import run_bass_kernel_spmd

B, N, D = 2048, 1024, 64
NCORES = 8
NS = N // NCORES          # 128 nets per core
PAIRS = NS // 2           # 64
MMW = 512                 # matmul moving free dim (one PSUM bank of fp32 out)

TWO_PI = float(2.0 * np.pi)
MAGIC = float(1.5 * 2 ** 23)
XLS = float(2.0 ** 11)    # x-lo scale

F32 = mybir.dt.float32
BF16 = mybir.dt.bfloat16
FP16 = mybir.dt.float16
F8E4 = mybir.dt.float8e4


def build_bass(repeat: int = 1):
    """Build the per-core Bass program (same NEFF on all 8 cores).

    repeat > 1 re-runs the whole main loop (idempotent writes) for timing.
    """
    nc = bacc.Bacc("TRN2", target_bir_lowering=False, debug=False,
                   num_devices=NCORES)
    xh_d = nc.dram_tensor("xh", [PAIRS, 128, B], FP16, kind="ExternalInput")
    xl_d = nc.dram_tensor("xl", [PAIRS, 128, B], F8E4, kind="ExternalInput")
    wh_d = nc.dram_tensor("wh", [128, PAIRS * 128], BF16,
                          kind="ExternalInput")
    wl_d = nc.dram_tensor("wl", [128, PAIRS * 128], BF16,
                          kind="ExternalInput")
    ws_d = nc.dram_tensor("ws", [128, PAIRS * 128], BF16,
                          kind="ExternalInput")
    b2_d = nc.dram_tensor("b2", [4, PAIRS * 128], BF16, kind="ExternalInput")
    yT_d = nc.dram_tensor("yT", [PAIRS, 128, B], BF16, kind="ExternalOutput")

    with tile.TileContext(nc) as tc:
        with (
            tc.tile_pool(name="wconst", bufs=1) as wc_pool,
            tc.tile_pool(name="xin", bufs=2) as x_pool,
            tc.tile_pool(name="tt", bufs=4) as t_pool,
            tc.tile_pool(name="dd", bufs=4) as d_pool,
            tc.tile_pool(name="oout", bufs=2) as o_pool,
            tc.tile_pool(name="ps", bufs=4, space="PSUM") as psum_pool,
        ):
            # --- constants (loaded once) ---
            wh_sb = wc_pool.tile([128, PAIRS * 128], BF16)
            wl_sb = wc_pool.tile([128, PAIRS * 128], BF16)
            ws_sb = wc_pool.tile([128, PAIRS * 128], BF16)
            for sb_t, d_t in ((wh_sb, wh_d), (wl_sb, wl_d), (ws_sb, ws_d)):
                for _c in range(4):
                    _w = PAIRS * 128 // 4
                    nc.scalar.dma_start(sb_t[:, _c * _w:(_c + 1) * _w],
                                        d_t[:, _c * _w:(_c + 1) * _w])
            b2_sb = wc_pool.tile([4, PAIRS * 128], BF16)
            nc.gpsimd.dma_start(b2_sb[:], b2_d[:])
            ones = wc_pool.tile([4, MMW], BF16)
            nc.gpsimd.memset(ones[:], 1.0)
            magic_sb = wc_pool.tile([128, 1], F32)
            nc.gpsimd.memset(magic_sb[:], MAGIC)

            # --- main loop (optionally wrapped in a HW loop for timing) ---
            import contextlib
            rep_ctx = tc.For_i(0, repeat, 1) if repeat > 1 else contextlib.nullcontext()
            PB = 4   # pairs batched per DMA transfer
            EW = 1024  # elementwise unit / PSUM tile width (2 banks)
            unit = 0
            with rep_ctx:
                for p0 in range(0, PAIRS, PB):
                    xh = x_pool.tile([128, PB * B], FP16, tag="xh")
                    nc.sync.dma_start(
                        xh[:].rearrange("p (a b) -> p a b", a=PB),
                        xh_d[p0:p0 + PB].rearrange("a p b -> p a b"))
                    xl = x_pool.tile([128, PB * B], F8E4, tag="xl")
                    nc.sync.dma_start(
                        xl[:].rearrange("p (a b) -> p a b", a=PB),
                        xl_d[p0:p0 + PB].rearrange("a p b -> p a b"))
                    ot = o_pool.tile([128, PB * B], BF16)
                    for a in range(PB):
                        p = p0 + a
                        cs = slice(p * 128, (p + 1) * 128)
                        d = d_pool.tile([128, B], FP16, tag="d")
                        for e in range(B // EW):
                            u = psum_pool.tile([128, EW], F32)
                            for h in range(EW // MMW):
                                lo = h * MMW
                                ms = slice(a * B + e * EW + lo,
                                           a * B + e * EW + lo + MMW)
                                nc.tensor.matmul(
                                    u[:, lo:lo + MMW], wh_sb[:, cs],
                                    xh[:, ms], start=True, stop=False)
                                nc.tensor.matmul(
                                    u[:, lo:lo + MMW], wl_sb[:, cs],
                                    xh[:, ms], start=False, stop=False)
                                nc.tensor.matmul(
                                    u[:, lo:lo + MMW], ws_sb[:, cs],
                                    xl[:, ms], start=False, stop=False)
                                nc.tensor.matmul(
                                    u[:, lo:lo + MMW], b2_sb[:, cs],
                                    ones[:], start=False, stop=True)
                            t = t_pool.tile([128, EW], F32, tag="t")
                            # balance the magic-round pass: 3 of 8 on DVE
                            if unit % 8 in (0, 3, 6):
                                nc.vector.tensor_scalar_add(
                                    t[:], u[:], MAGIC)
                            else:
                                nc.scalar.activation(
                                    t[:], u[:],
                                    mybir.ActivationFunctionType.Identity,
                                    bias=magic_sb[:], scale=1.0)
                            unit += 1
                            nc.vector.scalar_tensor_tensor(
                                d[:, e * EW:(e + 1) * EW], t[:], MAGIC, u[:],
                                op0=AluOpType.subtract,
                                op1=AluOpType.subtract)
                        nc.scalar.activation(
                            ot[:, a * B:(a + 1) * B], d[:],
                            mybir.ActivationFunctionType.Sin,
                            bias=0.0, scale=-TWO_PI)
                    nc.gpsimd.dma_start(
                        yT_d[p0:p0 + PB].rearrange("a p b -> p a b"),
                        ot[:].rearrange("p (a b) -> p a b", a=PB))
    nc.compile()
    return nc


def prep_inputs(x, weights, bias, omega):
    """Host-side layout prep -> list of 8 per-core input dicts."""
    bf16 = ml_dtypes.bfloat16
    f8e4 = ml_dtypes.float8_e4m3
    x3 = x.reshape(B, NCORES, NS, D)
    # xT_all[c, n, i, b] = x[b, c*128+n, i]; blocked for cache friendliness
    xT_all = np.empty((NCORES, NS, D, B), np.float32)
    BBLK = 128
    for b0 in range(0, B, BBLK):
        xT_all[:, :, :, b0:b0 + BBLK] = x3[b0:b0 + BBLK].transpose(1, 2, 3, 0)

    in_maps = []
    for c in range(NCORES):
        sl = slice(c * NS, (c + 1) * NS)
        sc = (omega[sl].astype(np.float64) / (2.0 * np.pi))  # [128]
        ws = (weights[sl].astype(np.float64)
              * sc[:, None, None]).astype(np.float32)        # [128, 64, 64]
        wT = np.ascontiguousarray(ws.transpose(0, 2, 1))     # [net, i, j]

        def blockdiag(wt):
            bd = np.zeros((128, PAIRS, 128), np.float32)
            bd[:D, :, :D] = wt[0::2].transpose(1, 0, 2)
            bd[D:, :, D:] = wt[1::2].transpose(1, 0, 2)
            return bd.reshape(128, PAIRS * 128)

        wbd = blockdiag(wT)
        wh = wbd.astype(bf16)
        wl = (wbd - wh.astype(np.float32)).astype(bf16)
        wsc = (wbd * np.float32(1.0 / XLS)).astype(bf16)

        bs = (bias[sl].astype(np.float64) * sc[:, None]).astype(np.float32)
        b_hi = bs.astype(bf16)
        b_lo = (bs - b_hi.astype(np.float32)).astype(bf16)
        b2 = np.zeros((PAIRS, 4, 128), bf16)
        b2[:, 0, :D] = b_hi[0::2]
        b2[:, 1, :D] = b_lo[0::2]
        b2[:, 2, D:] = b_hi[1::2]
        b2[:, 3, D:] = b_lo[1::2]
        b2_host = np.ascontiguousarray(
            b2.transpose(1, 0, 2).reshape(4, PAIRS * 128))

        xT_c = xT_all[c].reshape(PAIRS, 128, B)
        xh_c = xT_c.astype(np.float16)
        xl_c = ((xT_c - xh_c.astype(np.float32)) * np.float32(XLS)
                ).astype(f8e4)
        in_maps.append({
            "xh": np.ascontiguousarray(xh_c),
            "xl": np.ascontiguousarray(xl_c),
            "wh": np.ascontiguousarray(wh),
            "wl": np.ascontiguousarray(wl),
            "ws": np.ascontiguousarray(wsc),
            "b2": b2_host,
        })
    return in_maps


def assemble_output(results):
    """[8 cores] of yT bf16 [PAIRS, 128, B] -> full fp32 [B, N*D]."""
    out = np.empty((B, N * D), np.float32)
    for c in range(NCORES):
        yy = results[c]["yT"].reshape(NS * D, B)
        ov = out[:, c * NS * D:(c + 1) * NS * D]
        for b0 in range(0, B, 128):
            ov[b0:b0 + 128, :] = yy[:, b0:b0 + 128].T.astype(np.float32)
    return out


_NC_CACHE = {}


def kernel(x, weights, bias, omega):
    x = np.ascontiguousarray(x, np.float32)
    weights = np.ascontiguousarray(weights, np.float32)
    bias = np.ascontiguousarray(bias, np.float32)
    omega = np.ascontiguousarray(omega, np.float32)

    if "nc" not in _NC_CACHE:
        _NC_CACHE["nc"] = build_bass()
    nc = _NC_CACHE["nc"]
    in_maps = prep_inputs(x, weights, bias, omega)
    res = run_bass_kernel_spmd(nc, in_maps, core_ids=list(range(NCORES)))
    return assemble_output(res.results)
